# revision 8
# baseline (speedup 1.0000x reference)
"""GatedAttention TRN2 kernel — 8-core tensor-parallel (1 kv-head group per core).

Self-contained: host-side shard/layout prep + Bass/Tile kernel + gather.

Per-core dataflow (all device tensors feature-on-partition, "T" layouts):
  qkvT = W_c.T @ xT           (f32r matmuls, PSUM accumulation over 16 h-chunks)
  RMS scales via ones-selector matmuls (partition-dim sums), ln/exp on ACT
  RoPE on DVE with host-prefolded cos/sin tables (gain + rotate-half sign baked)
  scoresT[sj,si] per head, row-tiled head pairs on the PE array
  exp on ACT with per-partition scale = 0.125 * rsqrt(mean k^2)  (no max-sub:
  |scores*scale| <= 8 by Cauchy-Schwarz after RMS norm)
  P@V with V augmented by a ones column (M=65) -> fused softmax denominators
  out_partial = attnT_scaled.T @ Wo_c ; host sums the 8 partials.
"""
import math
import os
import sys
import numpy as np
import ml_dtypes

BF16 = ml_dtypes.bfloat16

H, NH, KVH, HD = 2048, 32, 8, 64
G = NH // KVH          # 4 q heads per core
S = 2048
EPS = 1e-6
THETA = 1000000.0
SCALE = 1.0 / math.sqrt(HD)
NCORES = 8
HC = H // 128          # 16 h-chunks
NB = S // 512          # 4 si-blocks
NJ = S // 128          # 16 sj-chunks

_BUILT = {}
LAST_EXEC_NS = None


# ---------------------------------------------------------------- host prep
def _host_prep(hidden_states, Wq, Wk, Wv, Wo, g_q, g_k):
    x = np.ascontiguousarray(np.asarray(hidden_states, np.float32).reshape(S, H))
    Wq = np.asarray(Wq, np.float32)
    Wk = np.asarray(Wk, np.float32)
    Wv = np.asarray(Wv, np.float32)
    Wo = np.asarray(Wo, np.float32)
    g_q = np.asarray(g_q, np.float32)
    g_k = np.asarray(g_k, np.float32)

    xT = np.ascontiguousarray(x.T).astype(BF16)

    inv_freq = 1.0 / (THETA ** (np.arange(0, HD, 2, dtype=np.float32) / HD))
    pos = np.arange(S, dtype=np.float32)
    emb = np.concatenate([pos[:, None] * inv_freq[None, :]] * 2, axis=-1)  # [S,64]
    cos = np.cos(emb).T.astype(np.float32)   # [64, S]
    sin = np.sin(emb).T.astype(np.float32)
    sign = np.where(np.arange(HD) < HD // 2, -1.0, 1.0).astype(np.float32)[:, None]
    cosq = np.ascontiguousarray(cos * g_q[:, None])
    sinq = np.ascontiguousarray(sin * sign * np.roll(g_q, -32)[:, None])
    cosk = np.ascontiguousarray(cos * g_k[:, None])
    sink = np.ascontiguousarray(sin * sign * np.roll(g_k, -32)[:, None])

    in_maps = []
    for c in range(NCORES):
        Wq_g = Wq[:, c * (G * HD + G):(c + 1) * (G * HD + G)]
        gpad = np.zeros((H, 64), np.float32)
        for p in range(2):
            for hh in range(2):
                gpad[:, 32 * p + hh] = Wq_g[:, G * HD + 2 * p + hh]
        W_c = np.ascontiguousarray(np.concatenate(
            [Wq_g[:, :G * HD],
             Wk[:, c * HD:(c + 1) * HD],
             Wv[:, c * HD:(c + 1) * HD],
             gpad], axis=1))                                   # [H, 448]
        Wo_c = np.ascontiguousarray(Wo[c * G * HD:(c + 1) * G * HD, :])  # [256,H]
        in_maps.append({"xT": xT, "W": W_c.astype(BF16), "Wo": Wo_c.astype(BF16),
                        "cosq": cosq, "sinq": sinq, "cosk": cosk, "sink": sink})
    return in_maps


# ---------------------------------------------------------------- bass build
def _build_nc():
    import concourse.bass as bass
    import concourse.mybir as mybir
    import concourse.tile as tile
    from concourse import bacc
    from concourse.masks import make_identity, make_upper_triangular

    dt = mybir.dt
    f32 = dt.float32
    bf16 = dt.bfloat16
    AF = mybir.ActivationFunctionType

    nc = bacc.Bacc("TRN2", target_bir_lowering=False, debug=False,
                   num_devices=NCORES)

    xT_d = nc.dram_tensor("xT", [H, S], bf16, kind="ExternalInput")
    W_d = nc.dram_tensor("W", [H, 448], bf16, kind="ExternalInput")
    Wo_d = nc.dram_tensor("Wo", [G * HD, H], bf16, kind="ExternalInput")
    cosq_d = nc.dram_tensor("cosq", [HD, S], f32, kind="ExternalInput")
    sinq_d = nc.dram_tensor("sinq", [HD, S], f32, kind="ExternalInput")
    cosk_d = nc.dram_tensor("cosk", [HD, S], f32, kind="ExternalInput")
    sink_d = nc.dram_tensor("sink", [HD, S], f32, kind="ExternalInput")
    out_d = nc.dram_tensor("out", [S, H], f32, kind="ExternalOutput")

    def bcast_rows(src, reps):
        """src [r, n] -> AP iterating [r, reps, n] (each row repeated reps
        times along the destination partition axis)."""
        return bass.AP(tensor=src.tensor, offset=src.offset,
                       ap=[src.ap[0], [0, reps], src.ap[1]])

    import contextlib
    with tile.TileContext(nc) as tc, contextlib.ExitStack() as ctx:
        const = ctx.enter_context(tc.tile_pool(name="const", bufs=1))
        big = ctx.enter_context(tc.tile_pool(name="big", bufs=1))
        xpool = ctx.enter_context(tc.tile_pool(name="xp", bufs=3))
        rawp = ctx.enter_context(tc.tile_pool(name="raw", bufs=2))
        tmpp = ctx.enter_context(tc.tile_pool(name="tmp", bufs=2))
        sqp = ctx.enter_context(tc.tile_pool(name="sq", bufs=2))
        bcp = ctx.enter_context(tc.tile_pool(name="bc", bufs=2))
        expp = ctx.enter_context(tc.tile_pool(name="expp", bufs=4))
        outs = ctx.enter_context(tc.tile_pool(name="outs", bufs=3))
        smal = ctx.enter_context(tc.tile_pool(name="smal", bufs=2))
        psum = ctx.enter_context(tc.tile_pool(name="ps", bufs=1, space="PSUM"))

        # ---------------- constants
        id64 = const.tile([64, 64], f32, tag="id64")
        make_identity(nc, id64)
        tri = const.tile([128, 128], bf16, tag="tri")
        make_upper_triangular(nc, tri, val=1.0, diag=True)
        ones = const.tile([128, 1], f32, tag="ones")
        nc.vector.memset(ones, 1.0)
        esel = const.tile([128, 2], f32, tag="esel")
        nc.vector.memset(esel, 0.0)
        nc.vector.memset(esel[0:64, 0:1], 1.0)
        nc.vector.memset(esel[64:128, 1:2], 1.0)
        eps_b = const.tile([128, 1], f32, tag="epsb")
        nc.vector.memset(eps_b, EPS)
        lb8_b = const.tile([128, 1], f32, tag="lb8b")
        nc.vector.memset(lb8_b, math.log(SCALE))

        # ---------------- resident weights / tables
        W_sb = big.tile([128, HC, 448], bf16, tag="W")
        nc.sync.dma_start(out=W_sb, in_=W_d.ap().rearrange(
            "(hc p) c -> p hc c", p=128))
        Wo_sb = big.tile([128, 2, H], bf16, tag="Wo")
        nc.sync.dma_start(out=Wo_sb, in_=Wo_d.ap().rearrange(
            "(cc p) h -> p cc h", p=128))

        def pair_table(src_d, tag):
            t = big.tile([128, S], f32, tag=tag, name=tag)
            src = src_d.ap()
            ap2 = bass.AP(tensor=src.tensor, offset=src.offset,
                          ap=[[0, 2]] + list(src.ap))
            nc.sync.dma_start(out=t, in_=ap2)
            return t

        cosq_sb = pair_table(cosq_d, "cosq")
        sinq_sb = pair_table(sinq_d, "sinq")
        cosk_sb = big.tile([64, S], f32, tag="cosk")
        nc.sync.dma_start(out=cosk_sb, in_=cosk_d[:, :])
        sink_sb = big.tile([64, S], f32, tag="sink")
        nc.sync.dma_start(out=sink_sb, in_=sink_d[:, :])

        # ---------------- persistent activations
        kk2 = big.tile([128, S], bf16, tag="kk2")
        v_sb = big.tile([128, NJ, 65], bf16, tag="v")
        nc.vector.memset(v_sb[:, :, 64:65], 1.0)
        rkT_sb = big.tile([128, NJ], f32, tag="rkT")

        for sib in range(NB):
            sp = slice(sib * 512, (sib + 1) * 512)

            # ======== QKV projection for this si-block
            ps_cc = [psum.tile([128, 512], f32, tag="qkv3", bufs=3,
                               name=f"pscc{cc}") for cc in range(3)]
            ps_g = psum.tile([64, 512], f32, tag="gate", bufs=1)
            for hc in range(HC):
                xt = xpool.tile([128, 512], bf16, tag="xt")
                nc.sync.dma_start(out=xt, in_=xT_d[hc * 128:(hc + 1) * 128, sp])
                st = (hc == 0)
                fin = (hc == HC - 1)
                for cc in range(3):
                    nc.tensor.matmul(ps_cc[cc][:],
                                     W_sb[:, hc, cc * 128:(cc + 1) * 128],
                                     xt, start=st, stop=fin)
                nc.tensor.matmul(ps_g[:], W_sb[:, hc, 384:448], xt,
                                 start=st, stop=fin)

            qr = [rawp.tile([128, 512], f32, tag=f"qr{p}", name=f"qr{p}")
                  for p in range(2)]
            kr = rawp.tile([64, 512], f32, tag="kr")
            vr = rawp.tile([64, 512], f32, tag="vr")
            for p in range(2):
                nc.scalar.copy(qr[p], ps_cc[p][:])
            nc.scalar.copy(kr, ps_cc[2][0:64, :])
            nc.scalar.copy(vr, ps_cc[2][64:128, :])
            # sigmoid(gate) = 1/(1+exp(-g)) per head pair
            sig_t = []
            for p in range(2):
                eg_t = smal.tile([2, 512], f32, tag="smA")
                nc.scalar.activation(eg_t, ps_g[32 * p:32 * p + 2, :], AF.Exp,
                                     scale=-1.0)
                tmp_sig = smal.tile([2, 512], f32, tag="smB")
                nc.vector.tensor_scalar_add(tmp_sig, eg_t, 1.0)
                sg = smal.tile([2, 512], f32, tag="sig", bufs=4, name="sg")
                nc.vector.reciprocal(sg, tmp_sig)
                sig_t.append(sg)

            # ======== RMS scales
            rqt = [smal.tile([2, 512], f32, tag="rqt", bufs=4, name="rqp")
                   for p in range(2)]
            for p in range(2):
                sq = sqp.tile([128, 512], f32, tag="sq")
                nc.vector.tensor_mul(sq, qr[p], qr[p])
                ps_rq = psum.tile([2, 512], f32, tag="sc", bufs=2, name="psrq")
                nc.tensor.matmul(ps_rq[:], esel, sq,
                                 start=True, stop=True)
                ln_q = smal.tile([2, 512], f32, tag="smB")
                nc.scalar.activation(ln_q, ps_rq[:], AF.Ln, bias=eps_b[0:2, :], scale=1.0 / HD)
                nc.scalar.activation(rqt[p], ln_q, AF.Exp, scale=-0.5)

            ksq = sqp.tile([64, 512], f32, tag="ksq")
            nc.vector.tensor_mul(ksq, kr, kr)
            ps_rk = psum.tile([128, 4], f32, tag="sc", bufs=2, name="psrk")
            for j in range(4):
                nc.tensor.matmul(ps_rk[:, j:j + 1],
                                 ksq[:, j * 128:(j + 1) * 128],
                                 ones[0:64, :], start=True, stop=True)
            ln_k = smal.tile([128, 4], f32, tag="smB")
            nc.scalar.activation(ln_k, ps_rk[:], AF.Ln, bias=eps_b, scale=1.0 / HD)
            nc.scalar.activation(rkT_sb[:, sib * 4:(sib + 1) * 4], ln_k,
                                 AF.Exp, bias=lb8_b, scale=-0.5)

            # ======== RoPE (+ rq fold for q)
            qf = [rawp.tile([128, 512], bf16, tag=f"qf{p}", name=f"qf{p}", bufs=2)
                  for p in range(2)]
            for p in range(2):
                rqb = bcp.tile([128, 512], f32, tag="rqb")
                nc.sync.dma_start(out=rqb, in_=bcast_rows(rqt[p], 64))
                t1 = tmpp.tile([128, 512], f32, tag="t1")
                nc.vector.tensor_mul(t1, qr[p], cosq_sb[:, sp])
                qs = tmpp.tile([128, 512], f32, tag="qs")
                for g in range(2):
                    b = g * 64
                    nc.sync.dma_start(out=qs[b:b + 32, :], in_=qr[p][b + 32:b + 64, :])
                    nc.sync.dma_start(out=qs[b + 32:b + 64, :], in_=qr[p][b:b + 32, :])
                t2 = tmpp.tile([128, 512], f32, tag="t2")
                nc.vector.tensor_mul(t2, qs, sinq_sb[:, sp])
                nc.vector.tensor_add(t2, t1, t2)
                nc.vector.tensor_mul(qf[p], t2, rqb)

            t1k = tmpp.tile([64, 512], f32, tag="t1")
            nc.vector.tensor_mul(t1k, kr, cosk_sb[:, sp])
            ks = tmpp.tile([64, 512], f32, tag="qs")
            nc.sync.dma_start(out=ks[0:32, :], in_=kr[32:64, :])
            nc.sync.dma_start(out=ks[32:64, :], in_=kr[0:32, :])
            t2k = tmpp.tile([64, 512], f32, tag="t2")
            nc.vector.tensor_mul(t2k, ks, sink_sb[:, sp])
            nc.vector.tensor_add(kk2[0:64, sp], t1k, t2k)
            nc.vector.tensor_copy(kk2[64:128, sp], kk2[0:64, sp])

            # ======== V transpose (token-major, raw)
            for j in range(4):
                J = sib * 4 + j
                ps_v = psum.tile([128, 64], f32, tag="sc", bufs=2, name="psv")
                nc.tensor.transpose(ps_v[:], vr[:, j * 128:(j + 1) * 128], id64)
                nc.scalar.copy(v_sb[:, J, 0:64], ps_v[:])

            # ======== attention for si-block B = sib
            B = sib
            at = [rawp.tile([128, 512], bf16, tag=f"at{p}", name=f"at{p}", bufs=2)
                  for p in range(2)]
            for p in range(2):
                ps_att = [psum.tile([128, 512], f32, tag="att", bufs=2,
                                    name=f"psatt{hh}") for hh in range(2)]
                for J in range(4 * B + 4):
                    off = max(0, (J - 4 * B) * 128)
                    ssp = slice(B * 512 + off, (B + 1) * 512)
                    ex = []
                    for hh in range(2):
                        rb = hh * 64
                        ps_s = psum.tile([128, 512], f32, tag="sc", bufs=2,
                                         name="pss")
                        nc.tensor.matmul(
                            ps_s[:, off:512],
                            kk2[rb:rb + 64, J * 128:(J + 1) * 128],
                            qf[p][rb:rb + 64, off:512],
                            start=True, stop=True,
                            tile_position=(rb, 0))
                        et = expp.tile([128, 512], bf16, tag="expT", bufs=5,
                                       name="et")
                        nc.scalar.activation(et[:, off:512], ps_s[:, off:512],
                                             AF.Exp, scale=rkT_sb[:, J:J + 1])
                        if off > 0 or J == 4 * B:
                            nc.vector.tensor_mul(et[:, off:off + 128],
                                                 et[:, off:off + 128], tri)
                        ex.append(et)
                    for hh in range(2):
                        nc.tensor.matmul(
                            ps_att[hh][0:65, off:512],
                            v_sb[:, J, :],
                            ex[hh][:, off:512],
                            start=(J == 0), stop=(J == 4 * B + 3))

                # denominators -> scale s = sigmoid(gate)/den
                den2 = smal.tile([2, 512], f32, tag="smA")
                for hh in range(2):
                    dh = smal.tile([1, 512], f32, tag="smB")
                    nc.scalar.copy(dh, ps_att[hh][64:65, :])
                    nc.sync.dma_start(out=bass.AP(
                        tensor=den2.tensor, offset=den2[hh:hh + 1, :].offset,
                        ap=den2[hh:hh + 1, :].ap), in_=dh)
                rden = smal.tile([2, 512], f32, tag="smB")
                nc.vector.reciprocal(rden, den2)
                s_t = smal.tile([2, 512], f32, tag="den4")
                nc.vector.tensor_mul(s_t, sig_t[p], rden)
                sbc = bcp.tile([128, 512], f32, tag="sbc")
                nc.sync.dma_start(out=sbc, in_=bcast_rows(s_t, 64))
                for hh in range(2):
                    rb = hh * 64
                    nc.vector.tensor_mul(at[p][rb:rb + 64, :],
                                         ps_att[hh][0:64, :], sbc[rb:rb + 64, :])

            # ======== output projection for this block's si-chunks
            for ss in range(4 * B, 4 * B + 4):
                ls = (ss - 4 * B) * 128
                for qtr in range(4):
                    ps_o = psum.tile([128, 512], f32, tag="qkv3", bufs=3,
                                     name="pso")
                    nc.tensor.matmul(ps_o[:], at[0][:, ls:ls + 128],
                                     Wo_sb[:, 0, qtr * 512:(qtr + 1) * 512],
                                     start=True, stop=False)
                    nc.tensor.matmul(ps_o[:], at[1][:, ls:ls + 128],
                                     Wo_sb[:, 1, qtr * 512:(qtr + 1) * 512],
                                     start=False, stop=True)
                    ot = outs.tile([128, 512], f32, tag="ot")
                    if qtr % 2 == 0:
                        nc.scalar.copy(ot, ps_o[:])
                    else:
                        nc.vector.tensor_copy(ot, ps_o[:])
                    nc.sync.dma_start(
                        out=out_d[ss * 128:(ss + 1) * 128, qtr * 512:(qtr + 1) * 512],
                        in_=ot)

    nc.compile()
    return nc


def _get_nc():
    if "nc" not in _BUILT:
        _BUILT["nc"] = _build_nc()
    return _BUILT["nc"]


# ---------------------------------------------------------------- entry point
def _install_ntff_hook():
    import types
    try:
        import antenv
        if "antenv.axon_hooks" in sys.modules:
            return True
        mod = types.ModuleType("antenv.axon_hooks")
        holder = [None]
        mod.set_axon_ntff_profile_hook = lambda h: holder.__setitem__(0, h)
        mod.get_axon_ntff_profile_hook = lambda: holder[0]
        sys.modules["antenv.axon_hooks"] = mod
        antenv.axon_hooks = mod
        from trn_agent_boot.trn_boot import _ntff_profile_via_ctypes
        hook = _ntff_profile_via_ctypes("/opt/axon/libaxon_pjrt.so")
        if hook is None:
            return False
        mod.set_axon_ntff_profile_hook(hook)
        return True
    except Exception:
        return False


def kernel(hidden_states, Wq, Wk, Wv, Wo, g_q, g_k):
    global LAST_EXEC_NS
    from concourse.bass_utils import run_bass_kernel_spmd

    in_maps = _host_prep(hidden_states, Wq, Wk, Wv, Wo, g_q, g_k)
    nc = _get_nc()
    trace = os.environ.get("KERNEL_TRACE", "0") == "1"
    if trace:
        trace = _install_ntff_hook()
    res = run_bass_kernel_spmd(nc, in_maps, list(range(NCORES)), trace=trace)
    LAST_EXEC_NS = res.exec_time_ns
    out = np.zeros((S, H), np.float32)
    for c in range(NCORES):
        out += res.results[c]["out"]
    return out.reshape(1, S, H).astype(np.float32)
